# revision 9
# baseline (speedup 1.0000x reference)
"""MoE routing kernel for 8 Trainium2 NeuronCores.

Problem: B=65536 tokens, shared Linear(512->256)+ReLU, then per-token expert
MLP Linear(256->100)+ReLU -> Linear(100->1), expert chosen by idx in [0,16).

Strategy (expert-parallel, host-side routing):
  - Host sorts tokens by expert. Experts 2c and 2c+1 go to core c, each in a
    fixed-capacity slot of C tokens (C = max expert count rounded up to 128),
    padded with token 0 (padding outputs are computed then discarded).
  - Host pre-transposes x to [512, TOK] bf16 per core so the contraction dim
    (IN_DIM) lands on SBUF partitions: all three GEMMs then chain on-chip with
    no transposes (layer-1 out [hid, tok] feeds layer-2, which feeds layer-3).
  - Device: per group of <=512 tokens: one DMA of x columns, 8 accumulating
    matmuls (512-dim contraction, 2 hid chunks) + bias/ReLU on ScalarE,
    2 matmuls for expert FC1 + bias/ReLU, 1 matmul for FC2 + bias, DMA out.
  - Weights (tiny) live resident in SBUF in bf16; PSUM accumulates fp32.
"""

import math
import os
import sys

import numpy as np

for _p in ("/opt/trn_rl_repo", "/opt/pypackages"):
    if _p not in sys.path and os.path.isdir(_p):
        sys.path.append(_p)

import ml_dtypes

BF16 = ml_dtypes.bfloat16

B, IN_DIM, HID, EXP_HID, OUT_DIM, N_EXP = 65536, 512, 256, 100, 1, 16
N_CORES = 8
GROUP = 512  # tokens per matmul group (= PSUM bank free-dim in fp32)

_PROGRAM_CACHE = {}


def _build_program(C: int):
    """Build (and cache) the Bass program for per-expert-slot capacity C."""
    import concourse.bass as bass
    import concourse.mybir as mybir
    import concourse.tile as tile
    from concourse import bacc

    TOK = 2 * C
    f32 = mybir.dt.float32
    bf16 = mybir.dt.bfloat16
    AF = mybir.ActivationFunctionType

    nc = bacc.Bacc("TRN2", target_bir_lowering=False, debug=False)

    # x pre-blocked on host: xg[g, p, kc*512+t] = x[token off_g+t, kc*128+p]
    n_groups = 2 * ((C + GROUP - 1) // GROUP)
    xg_d = nc.dram_tensor(
        "xg", [n_groups, 128, 4 * GROUP], bf16, kind="ExternalInput"
    ).ap()
    ws_d = nc.dram_tensor("ws", [4, 128, HID], bf16, kind="ExternalInput").ap()
    bs_d = nc.dram_tensor("bs", [128, 2], f32, kind="ExternalInput").ap()
    # w1 padded to 128 output cols (100 real) so FWL kicks in on LDWEIGHTS
    w1_d = nc.dram_tensor("w1", [2, 2, 128, 128], bf16, kind="ExternalInput").ap()
    # b1 rows 0..99 = b1[e]; rows 100..127 = 1.0 so relu(0 + 1) makes a ones
    # row block that w2's bias row consumes (fc2 bias folded into the matmul)
    b1_d = nc.dram_tensor("b1", [128, 2], f32, kind="ExternalInput").ap()
    # w2 rows 0..99 = W2[e,:,0], row 100 = b2[e], rows 101..127 = 0
    w2_d = nc.dram_tensor("w2", [128, 2], bf16, kind="ExternalInput").ap()
    out_d = nc.dram_tensor("out", [1, TOK], f32, kind="ExternalOutput").ap()

    # group schedule: (expert_slot, token_offset, ntok)
    groups = []
    for slot in range(2):
        off = slot * C
        while off < (slot + 1) * C:
            n = min(GROUP, (slot + 1) * C - off)
            groups.append((slot, off, n))
            off += n

    ALU = mybir.AluOpType

    with tile.TileContext(nc) as tc:
        with (
            tc.tile_pool(name="const", bufs=1) as const,
            tc.tile_pool(name="xp", bufs=6) as xp,
            tc.tile_pool(name="hp", bufs=3) as hp,
            tc.tile_pool(name="h1p", bufs=3) as h1p,
            tc.tile_pool(name="ps1", bufs=3, space="PSUM") as ps1,
            tc.tile_pool(name="ps2", bufs=3, space="PSUM") as ps2,
            tc.tile_pool(name="ps3", bufs=2, space="PSUM") as ps3,
        ):
            # consts go on the gpsimd queue so the sync queue streams x
            # from instruction 0
            ws_sb = const.tile([128, 4, HID], bf16)
            for kc in range(4):
                nc.gpsimd.dma_start(ws_sb[:, kc, :], ws_d[kc])
            bs_sb = const.tile([128, 2], f32)
            nc.gpsimd.dma_start(bs_sb[:, :], bs_d[:, :])
            w1_sb = const.tile([128, 2, 2, 128], bf16)
            for e in range(2):
                for kc in range(2):
                    nc.gpsimd.dma_start(w1_sb[:, e, kc, :], w1_d[e, kc])
            b1_sb = const.tile([128, 2], f32)
            nc.gpsimd.dma_start(b1_sb[:, :], b1_d[:, :])
            w2_sb = const.tile([128, 2], bf16)
            nc.gpsimd.dma_start(w2_sb[:, :], w2_d[:, :])
            o_all = const.tile([1, TOK], f32)

            # PE warm-up: ~16 dummy matmuls while the first x DMAs are in
            # flight, so the HAM clock gate is already at 8/8 when real
            # matmuls start. Results are never read.
            warm_w = const.tile([128, 128], bf16)
            nc.vector.memset(warm_w[:, :], 0.0)
            warm_p = ps1.tile([128, GROUP], f32, tag="p1", name="warm_p")
            for _ in range(40):
                nc.tensor.matmul(
                    warm_p[:, :128], warm_w[:, :], warm_w[:, :], start=True, stop=True
                )

            for g, (e, off, n) in enumerate(groups):
                x_sb = xp.tile([128, 4, GROUP], bf16, tag="x")
                if n == GROUP:
                    nc.sync.dma_start(
                        x_sb.rearrange("p c t -> p (c t)"), xg_d[g]
                    )
                else:
                    nc.sync.dma_start(
                        x_sb[:, :, :n],
                        xg_d[g].rearrange("p (c t) -> p c t", c=4)[:, :, :n],
                    )

                h_sb = hp.tile([128, 2, GROUP], bf16, tag="h")
                for hc in range(2):
                    p1 = ps1.tile([128, GROUP], f32, tag="p1")
                    for kc in range(4):
                        nc.tensor.matmul(
                            p1[:, :n],
                            ws_sb[:, kc, hc * 128 : (hc + 1) * 128],
                            x_sb[:, kc, :n],
                            start=(kc == 0),
                            stop=(kc == 3),
                        )
                    # h = relu(psum + bs): hc0 on VectorE, hc1 on ScalarE
                    if hc == 0:
                        nc.vector.tensor_scalar(
                            h_sb[:, hc, :n],
                            p1[:, :n],
                            bs_sb[:, hc : hc + 1],
                            0.0,
                            ALU.add,
                            ALU.max,
                        )
                    else:
                        nc.scalar.activation(
                            h_sb[:, hc, :n],
                            p1[:, :n],
                            AF.Relu,
                            bias=bs_sb[:, hc : hc + 1],
                        )

                p2 = ps2.tile([128, GROUP], f32, tag="p2")
                for kc in range(2):
                    nc.tensor.matmul(
                        p2[:, :n],
                        w1_sb[:, e, kc, :],
                        h_sb[:, kc, :n],
                        start=(kc == 0),
                        stop=(kc == 1),
                    )
                # h1 rows 0..99 = relu(psum + b1); rows 100..127 = relu(0+1) = 1
                h1_sb = h1p.tile([128, GROUP], bf16, tag="h1")
                nc.vector.tensor_scalar(
                    h1_sb[:, :n],
                    p2[:, :n],
                    b1_sb[:, e : e + 1],
                    0.0,
                    ALU.add,
                    ALU.max,
                )

                p3 = ps3.tile([1, GROUP], f32, tag="p3")
                nc.tensor.matmul(
                    p3[:, :n],
                    w2_sb[:, e : e + 1],
                    h1_sb[:, :n],
                    start=True,
                    stop=True,
                )
                nc.scalar.copy(o_all[:, off : off + n], p3[:, :n])

            nc.sync.dma_start(out_d[:, :], o_all[:, :])

    nc.compile()
    return nc


def _get_program(C: int):
    if C not in _PROGRAM_CACHE:
        _PROGRAM_CACHE[C] = _build_program(C)
    return _PROGRAM_CACHE[C]


def kernel(x, idx, Ws, bs, W1, b1, W2, b2, _trace=False, _result_box=None):
    from concourse.bass_utils import run_bass_kernel_spmd

    x = np.asarray(x)
    idx = np.asarray(idx).astype(np.int64)
    Ws = np.asarray(Ws, dtype=np.float32)
    bs = np.asarray(bs, dtype=np.float32)
    W1 = np.asarray(W1, dtype=np.float32)
    b1 = np.asarray(b1, dtype=np.float32)
    W2 = np.asarray(W2, dtype=np.float32)
    b2 = np.asarray(b2, dtype=np.float32)

    counts = np.bincount(idx, minlength=N_EXP)
    C = max(GROUP, int(math.ceil(counts.max() / 128) * 128))
    TOK = 2 * C
    nc = _get_program(C)

    order = np.argsort(idx, kind="stable")
    bounds = np.zeros(N_EXP + 1, dtype=np.int64)
    np.cumsum(counts, out=bounds[1:])
    tok_by_expert = [order[bounds[e] : bounds[e + 1]] for e in range(N_EXP)]

    # shared-layer weights, chunked for the device (same for every core)
    ws_host = np.ascontiguousarray(Ws.reshape(4, 128, HID)).astype(BF16)
    bs_host = np.ascontiguousarray(bs.reshape(2, 128).T).astype(np.float32)

    x_bf = x.astype(BF16)
    in_maps = []
    core_tokens = []
    for c in range(N_CORES):
        ea, eb = 2 * c, 2 * c + 1
        toks = np.zeros(TOK, dtype=np.int64)
        toks[: counts[ea]] = tok_by_expert[ea]
        toks[C : C + counts[eb]] = tok_by_expert[eb]
        core_tokens.append(toks)

        # per-group contiguous blocks: xg[g, p, kc*512+t] = x[toks[g*512+t], kc*128+p]
        n_groups = 2 * ((C + GROUP - 1) // GROUP)
        toks_p = np.zeros(n_groups * GROUP, dtype=np.int64)
        gp = (C + GROUP - 1) // GROUP  # groups per slot
        for slot in range(2):
            toks_p[slot * gp * GROUP : slot * gp * GROUP + C] = toks[
                slot * C : (slot + 1) * C
            ]
        xg = np.ascontiguousarray(
            x_bf[toks_p].reshape(n_groups, GROUP, 4, 128).transpose(0, 3, 2, 1)
        ).reshape(n_groups, 128, 4 * GROUP)

        w1_pair = np.zeros((2, 2, 128, 128), dtype=BF16)
        w1_pair[:, :, :, :EXP_HID] = W1[[ea, eb]].reshape(2, 2, 128, EXP_HID).astype(BF16)
        b1_pair = np.ones((128, 2), dtype=np.float32)
        b1_pair[:EXP_HID] = b1[[ea, eb]].T
        w2_pair = np.zeros((128, 2), dtype=BF16)
        w2_pair[:EXP_HID] = W2[[ea, eb], :, 0].T.astype(BF16)
        w2_pair[EXP_HID] = b2[[ea, eb], 0].astype(BF16)

        in_maps.append(
            {
                "xg": xg,
                "ws": ws_host,
                "bs": bs_host,
                "w1": w1_pair,
                "b1": b1_pair,
                "w2": w2_pair,
            }
        )

    res = run_bass_kernel_spmd(
        nc,
        in_maps,
        core_ids=list(range(N_CORES)),
        trace=_trace,
        **({"trace_cores": [0]} if _trace else {}),
    )
    if _result_box is not None:
        _result_box.append(res)

    out = np.zeros((B, OUT_DIM), dtype=np.float32)
    for c in range(N_CORES):
        ea, eb = 2 * c, 2 * c + 1
        oc = res.results[c]["out"][0]  # [TOK] f32
        out[core_tokens[c][: counts[ea]], 0] = oc[: counts[ea]]
        out[core_tokens[c][C : C + counts[eb]], 0] = oc[C : C + counts[eb]]
    return out


# revision 11
# speedup vs baseline: 1.0572x; 1.0572x over previous
"""MoE routing kernel for 8 Trainium2 NeuronCores.

Problem: B=65536 tokens, shared Linear(512->256)+ReLU, then per-token expert
MLP Linear(256->100)+ReLU -> Linear(100->1), expert chosen by idx in [0,16).

Strategy (expert-parallel, host-side routing):
  - Host sorts tokens by expert. Experts 2c and 2c+1 go to core c, each in a
    fixed-capacity slot of C tokens (C = max expert count rounded up to 128),
    padded with token 0 (padding outputs are computed then discarded).
  - Host pre-transposes x to [512, TOK] bf16 per core so the contraction dim
    (IN_DIM) lands on SBUF partitions: all three GEMMs then chain on-chip with
    no transposes (layer-1 out [hid, tok] feeds layer-2, which feeds layer-3).
  - Device: per group of <=512 tokens: one DMA of x columns, 8 accumulating
    matmuls (512-dim contraction, 2 hid chunks) + bias/ReLU on ScalarE,
    2 matmuls for expert FC1 + bias/ReLU, 1 matmul for FC2 + bias, DMA out.
  - Weights (tiny) live resident in SBUF in bf16; PSUM accumulates fp32.
"""

import math
import os
import sys

import numpy as np

for _p in ("/opt/trn_rl_repo", "/opt/pypackages"):
    if _p not in sys.path and os.path.isdir(_p):
        sys.path.append(_p)

import ml_dtypes

BF16 = ml_dtypes.bfloat16

B, IN_DIM, HID, EXP_HID, OUT_DIM, N_EXP = 65536, 512, 256, 100, 1, 16
N_CORES = 8
GROUP = 512  # tokens per matmul group (= PSUM bank free-dim in fp32)

_PROGRAM_CACHE = {}


def _build_program(C: int):
    """Build (and cache) the Bass program for per-expert-slot capacity C."""
    import concourse.bass as bass
    import concourse.mybir as mybir
    import concourse.tile as tile
    from concourse import bacc

    TOK = 2 * C
    f32 = mybir.dt.float32
    bf16 = mybir.dt.bfloat16
    AF = mybir.ActivationFunctionType

    nc = bacc.Bacc("TRN2", target_bir_lowering=False, debug=False)

    # x pre-blocked on host: xg[g, p, kc*512+t] = x[token off_g+t, kc*128+p]
    n_groups = 2 * ((C + GROUP - 1) // GROUP)
    xg_d = nc.dram_tensor(
        "xg", [n_groups, 128, 4 * GROUP], bf16, kind="ExternalInput"
    ).ap()
    ws_d = nc.dram_tensor("ws", [4, 128, HID], bf16, kind="ExternalInput").ap()
    bs_d = nc.dram_tensor("bs", [128, 2], f32, kind="ExternalInput").ap()
    # w1 padded to 128 output cols (100 real) so FWL kicks in on LDWEIGHTS
    w1_d = nc.dram_tensor("w1", [2, 2, 128, 128], bf16, kind="ExternalInput").ap()
    # b1 rows 0..99 = b1[e]; rows 100..127 = 1.0 so relu(0 + 1) makes a ones
    # row block that w2's bias row consumes (fc2 bias folded into the matmul)
    b1_d = nc.dram_tensor("b1", [128, 2], f32, kind="ExternalInput").ap()
    # w2 rows 0..99 = W2[e,:,0], row 100 = b2[e], rows 101..127 = 0
    w2_d = nc.dram_tensor("w2", [128, 2], bf16, kind="ExternalInput").ap()
    out_d = nc.dram_tensor("out", [1, TOK], f32, kind="ExternalOutput").ap()

    # group schedule: (expert_slot, token_offset, ntok)
    groups = []
    for slot in range(2):
        off = slot * C
        while off < (slot + 1) * C:
            n = min(GROUP, (slot + 1) * C - off)
            groups.append((slot, off, n))
            off += n

    ALU = mybir.AluOpType

    with tile.TileContext(nc) as tc:
        with (
            tc.tile_pool(name="const", bufs=1) as const,
            tc.tile_pool(name="xp", bufs=6) as xp,
            tc.tile_pool(name="hp", bufs=3) as hp,
            tc.tile_pool(name="h1p", bufs=3) as h1p,
            tc.tile_pool(name="ps1", bufs=3, space="PSUM") as ps1,
            tc.tile_pool(name="ps2", bufs=3, space="PSUM") as ps2,
            tc.tile_pool(name="ps3", bufs=2, space="PSUM") as ps3,
        ):
            # All DMAs ride the sync (HWDGE) queue. Interleave the const
            # loads between the first x-group issues so layer-1 weights are
            # resident before x arrives and x prefetch is never blocked.
            ws_sb = const.tile([128, 4, HID], bf16)
            bs_sb = const.tile([128, 2], f32)
            w1_sb = const.tile([128, 2, 2, 128], bf16)
            b1_sb = const.tile([128, 2], f32)
            w2_sb = const.tile([128, 2], bf16)
            o_all = const.tile([1, TOK], f32)
            x_tiles = []

            def issue_x(g):
                if g >= len(groups) or g < len(x_tiles):
                    return
                x_sb = xp.tile([128, 4, GROUP], bf16, tag="x", name=f"x_sb{g}")
                n = groups[g][2]
                if n == GROUP:
                    nc.sync.dma_start(x_sb.rearrange("p c t -> p (c t)"), xg_d[g])
                else:
                    nc.sync.dma_start(
                        x_sb[:, :, :n],
                        xg_d[g].rearrange("p (c t) -> p c t", c=4)[:, :, :n],
                    )
                x_tiles.append(x_sb)

            nc.sync.dma_start(ws_sb[:, 0, :], ws_d[0])
            nc.sync.dma_start(ws_sb[:, 1, :], ws_d[1])
            issue_x(0)
            nc.sync.dma_start(ws_sb[:, 2, :], ws_d[2])
            nc.sync.dma_start(ws_sb[:, 3, :], ws_d[3])
            nc.sync.dma_start(bs_sb[:, :], bs_d[:, :])
            issue_x(1)
            for e in range(2):
                for kc in range(2):
                    nc.sync.dma_start(w1_sb[:, e, kc, :], w1_d[e, kc])
            issue_x(2)
            nc.sync.dma_start(b1_sb[:, :], b1_d[:, :])
            nc.sync.dma_start(w2_sb[:, :], w2_d[:, :])
            for g in range(len(groups)):
                issue_x(g)

            # PE warm-up: dummy matmuls while the first x DMAs are in
            # flight, so the HAM clock gate is already 8/8 when real
            # matmuls start. Results are never read.
            warm_w = const.tile([128, 128], bf16)
            nc.vector.memset(warm_w[:, :], 0.0)
            warm_p = ps1.tile([128, GROUP], f32, tag="p1", name="warm_p")
            for _ in range(40):
                nc.tensor.matmul(
                    warm_p[:, :128], warm_w[:, :], warm_w[:, :], start=True, stop=True
                )

            for g, (e, off, n) in enumerate(groups):
                x_sb = x_tiles[g]

                h_sb = hp.tile([128, 2, GROUP], bf16, tag="h")
                for hc in range(2):
                    p1 = ps1.tile([128, GROUP], f32, tag="p1")
                    for kc in range(4):
                        nc.tensor.matmul(
                            p1[:, :n],
                            ws_sb[:, kc, hc * 128 : (hc + 1) * 128],
                            x_sb[:, kc, :n],
                            start=(kc == 0),
                            stop=(kc == 3),
                        )
                    # h = relu(psum + bs): hc0 on VectorE, hc1 on ScalarE
                    if hc == 0:
                        nc.vector.tensor_scalar(
                            h_sb[:, hc, :n],
                            p1[:, :n],
                            bs_sb[:, hc : hc + 1],
                            0.0,
                            ALU.add,
                            ALU.max,
                        )
                    else:
                        nc.scalar.activation(
                            h_sb[:, hc, :n],
                            p1[:, :n],
                            AF.Relu,
                            bias=bs_sb[:, hc : hc + 1],
                        )

                p2 = ps2.tile([128, GROUP], f32, tag="p2")
                for kc in range(2):
                    nc.tensor.matmul(
                        p2[:, :n],
                        w1_sb[:, e, kc, :],
                        h_sb[:, kc, :n],
                        start=(kc == 0),
                        stop=(kc == 1),
                    )
                # h1 rows 0..99 = relu(psum + b1); rows 100..127 = relu(0+1) = 1
                h1_sb = h1p.tile([128, GROUP], bf16, tag="h1")
                nc.vector.tensor_scalar(
                    h1_sb[:, :n],
                    p2[:, :n],
                    b1_sb[:, e : e + 1],
                    0.0,
                    ALU.add,
                    ALU.max,
                )

                p3 = ps3.tile([1, GROUP], f32, tag="p3")
                nc.tensor.matmul(
                    p3[:, :n],
                    w2_sb[:, e : e + 1],
                    h1_sb[:, :n],
                    start=True,
                    stop=True,
                )
                nc.scalar.copy(o_all[:, off : off + n], p3[:, :n])

            nc.sync.dma_start(out_d[:, :], o_all[:, :])

    nc.compile()
    return nc


def _get_program(C: int):
    if C not in _PROGRAM_CACHE:
        _PROGRAM_CACHE[C] = _build_program(C)
    return _PROGRAM_CACHE[C]


def kernel(x, idx, Ws, bs, W1, b1, W2, b2, _trace=False, _result_box=None):
    from concourse.bass_utils import run_bass_kernel_spmd

    x = np.asarray(x)
    idx = np.asarray(idx).astype(np.int64)
    Ws = np.asarray(Ws, dtype=np.float32)
    bs = np.asarray(bs, dtype=np.float32)
    W1 = np.asarray(W1, dtype=np.float32)
    b1 = np.asarray(b1, dtype=np.float32)
    W2 = np.asarray(W2, dtype=np.float32)
    b2 = np.asarray(b2, dtype=np.float32)

    counts = np.bincount(idx, minlength=N_EXP)
    C = max(GROUP, int(math.ceil(counts.max() / 128) * 128))
    TOK = 2 * C
    nc = _get_program(C)

    order = np.argsort(idx, kind="stable")
    bounds = np.zeros(N_EXP + 1, dtype=np.int64)
    np.cumsum(counts, out=bounds[1:])
    tok_by_expert = [order[bounds[e] : bounds[e + 1]] for e in range(N_EXP)]

    # shared-layer weights, chunked for the device (same for every core)
    ws_host = np.ascontiguousarray(Ws.reshape(4, 128, HID)).astype(BF16)
    bs_host = np.ascontiguousarray(bs.reshape(2, 128).T).astype(np.float32)

    x_bf = x.astype(BF16)
    in_maps = []
    core_tokens = []
    for c in range(N_CORES):
        ea, eb = 2 * c, 2 * c + 1
        toks = np.zeros(TOK, dtype=np.int64)
        toks[: counts[ea]] = tok_by_expert[ea]
        toks[C : C + counts[eb]] = tok_by_expert[eb]
        core_tokens.append(toks)

        # per-group contiguous blocks: xg[g, p, kc*512+t] = x[toks[g*512+t], kc*128+p]
        n_groups = 2 * ((C + GROUP - 1) // GROUP)
        toks_p = np.zeros(n_groups * GROUP, dtype=np.int64)
        gp = (C + GROUP - 1) // GROUP  # groups per slot
        for slot in range(2):
            toks_p[slot * gp * GROUP : slot * gp * GROUP + C] = toks[
                slot * C : (slot + 1) * C
            ]
        xg = np.ascontiguousarray(
            x_bf[toks_p].reshape(n_groups, GROUP, 4, 128).transpose(0, 3, 2, 1)
        ).reshape(n_groups, 128, 4 * GROUP)

        w1_pair = np.zeros((2, 2, 128, 128), dtype=BF16)
        w1_pair[:, :, :, :EXP_HID] = W1[[ea, eb]].reshape(2, 2, 128, EXP_HID).astype(BF16)
        b1_pair = np.ones((128, 2), dtype=np.float32)
        b1_pair[:EXP_HID] = b1[[ea, eb]].T
        w2_pair = np.zeros((128, 2), dtype=BF16)
        w2_pair[:EXP_HID] = W2[[ea, eb], :, 0].T.astype(BF16)
        w2_pair[EXP_HID] = b2[[ea, eb], 0].astype(BF16)

        in_maps.append(
            {
                "xg": xg,
                "ws": ws_host,
                "bs": bs_host,
                "w1": w1_pair,
                "b1": b1_pair,
                "w2": w2_pair,
            }
        )

    res = run_bass_kernel_spmd(
        nc,
        in_maps,
        core_ids=list(range(N_CORES)),
        trace=_trace,
        **({"trace_cores": [0]} if _trace else {}),
    )
    if _result_box is not None:
        _result_box.append(res)

    out = np.zeros((B, OUT_DIM), dtype=np.float32)
    for c in range(N_CORES):
        ea, eb = 2 * c, 2 * c + 1
        oc = res.results[c]["out"][0]  # [TOK] f32
        out[core_tokens[c][: counts[ea]], 0] = oc[: counts[ea]]
        out[core_tokens[c][C : C + counts[eb]], 0] = oc[C : C + counts[eb]]
    return out
